# revision 1
# baseline (speedup 1.0000x reference)
"""MinLSTM fused kernel for Trainium2 (8 NeuronCores, batch-parallel).

Contract: kernel(**inputs) takes the FULL inputs from setup_inputs()
  x    [8, 4096, 1024] f32
  w_gh [1024, 3072]    f32
and returns the FULL output next_cell [8, 4096, 1024] f32.

Strategy
--------
Data-parallel over batch: core b computes batch b. Per core:
  g = x[b] @ w_gh  (fp16 operands, fp32 PSUM accumulate; x centered at 0 and
  w scaled by 32 to stay in fp16 normal range — both undone exactly via the
  ScalarE activation's per-partition bias / scale arguments)
then the minLSTM recurrence in linear domain (no log/exp):
  f = sigmoid(g_f); i = sigmoid(g_i); th = g_h
  num = i+eps; s = (f+eps)+num
  a = 1 + (num+eps)/f          == exp(log_f_prime)
  b = s*th/num                 == exp(log_state)   (the a*eps term is < 4e-8
                                                    relative and is dropped)
  P = cumprod_t(a)             (VectorE tensor_tensor_scan along free dim)
  out = P*b
Layout: channels on partitions, T along the free dimension ([H, T] tiles), so
the T-scan maps onto the hardware scan. Device output is [H, T] per core; the
host transposes back when reassembling the [B, T, H] result.
"""

from contextlib import ExitStack

import numpy as np

import concourse.tile as tile
from concourse import bacc, mybir

F32 = mybir.dt.float32
F16 = mybir.dt.float16
AF = mybir.ActivationFunctionType
OP = mybir.AluOpType

B, T, H = 8, 4096, 1024
TC = 512
EPS = 1e-8
WSCALE = 32.0
N_CORES = 8

_prog_cache = {}


def _build():
    nc = bacc.Bacc("TRN2", target_bir_lowering=False, debug=False)
    KB = H // 128
    CB = H // 128
    NB = T // TC
    H3 = 3 * H
    MB = H3 // 128
    inv_ws = float(1.0 / WSCALE)

    xT = nc.dram_tensor("xT", [H, T], F16, kind="ExternalInput")
    w = nc.dram_tensor("w", [H, H3], F16, kind="ExternalInput")
    bias = nc.dram_tensor("bias", [H3], F32, kind="ExternalInput")
    out = nc.dram_tensor("out", [H, T], F32, kind="ExternalOutput")

    with ExitStack() as ctx:
        tc = ctx.enter_context(tile.TileContext(nc))
        singles = ctx.enter_context(tc.tile_pool(name="singles", bufs=1))
        xin = ctx.enter_context(tc.tile_pool(name="xin", bufs=3))
        ps = ctx.enter_context(tc.tile_pool(name="ps", bufs=2, space="PSUM"))
        ew = ctx.enter_context(tc.tile_pool(name="ew", bufs=3))
        pp = ctx.enter_context(tc.tile_pool(name="pp", bufs=2))
        outp = ctx.enter_context(tc.tile_pool(name="outp", bufs=3))

        w_sb = singles.tile([128, KB, H3], F16)
        nc.sync.dma_start(out=w_sb, in_=w.rearrange("(k p) m -> p k m", p=128))
        bias_sb = singles.tile([128, MB], F32)
        nc.sync.dma_start(out=bias_sb, in_=bias.rearrange("(j p) -> p j", p=128))

        xTr = xT.rearrange("(k p) t -> p k t", p=128)
        prevP = [None] * CB
        for n in range(NB):
            tsl = slice(n * TC, (n + 1) * TC)
            x_t = xin.tile([128, KB, TC], F16, tag="x")
            nc.sync.dma_start(out=x_t, in_=xTr[:, :, tsl])

            for c in range(CB):
                psf = ps.tile([128, TC], F32, tag="pf")
                psi = ps.tile([128, TC], F32, tag="pi")
                psh = ps.tile([128, TC], F32, tag="ph")
                for j, pt in ((c, psf), (CB + c, psi), (2 * CB + c, psh)):
                    for k in range(KB):
                        nc.tensor.matmul(pt,
                                         lhsT=w_sb[:, k, j * 128:(j + 1) * 128],
                                         rhs=x_t[:, k, :],
                                         start=(k == 0), stop=(k == KB - 1))

                f_t = ew.tile([128, TC], F32, tag="f")
                i_t = ew.tile([128, TC], F32, tag="i")
                th_t = ew.tile([128, TC], F32, tag="th")
                nc.scalar.activation(f_t, psf, AF.Sigmoid,
                                     bias=bias_sb[:, c:c + 1], scale=inv_ws)
                nc.scalar.activation(i_t, psi, AF.Sigmoid,
                                     bias=bias_sb[:, CB + c:CB + c + 1], scale=inv_ws)
                nc.scalar.activation(th_t, psh, AF.Identity,
                                     bias=bias_sb[:, 2 * CB + c:2 * CB + c + 1],
                                     scale=inv_ws)

                num_t = ew.tile([128, TC], F32, tag="num")
                nc.vector.tensor_scalar_add(num_t, i_t, EPS)
                s_t = ew.tile([128, TC], F32, tag="s")
                nc.vector.scalar_tensor_tensor(s_t, in0=f_t, scalar=EPS, in1=num_t,
                                               op0=OP.add, op1=OP.add)
                rden_t = ew.tile([128, TC], F32, tag="rden")
                nc.vector.reciprocal_approx_fast(rden_t, f_t)
                rnum_t = ew.tile([128, TC], F32, tag="rnum")
                nc.vector.reciprocal_approx_fast(rnum_t, num_t)
                u_t = ew.tile([128, TC], F32, tag="u")
                nc.vector.tensor_tensor(u_t, th_t, rnum_t, OP.mult)
                b_t = ew.tile([128, TC], F32, tag="b")
                nc.vector.tensor_tensor(b_t, s_t, u_t, OP.mult)

                # a = 1 + (num+eps)*rden rather than s*rden: the approx
                # reciprocal is ~1e-6 biased low and the scan integrates any
                # bias on a over all 4096 steps; keeping the reciprocal in the
                # small term makes that contribution negligible.
                t1_t = ew.tile([128, TC], F32, tag="t1")
                nc.vector.scalar_tensor_tensor(t1_t, in0=num_t, scalar=EPS,
                                               in1=rden_t, op0=OP.add, op1=OP.mult)
                a_t = ew.tile([128, TC], F32, tag="a")
                nc.vector.tensor_scalar_add(a_t, t1_t, 1.0)

                P_t = pp.tile([128, TC], F32, tag=f"P{c}")
                init = 1.0 if n == 0 else prevP[c][:, TC - 1:TC]
                nc.vector.tensor_tensor_scan(P_t, a_t, a_t, initial=init,
                                             op0=OP.mult, op1=OP.bypass)
                prevP[c] = P_t

                o_t = outp.tile([128, TC], F32, tag="o")
                nc.vector.tensor_tensor(o_t, P_t, b_t, OP.mult)
                nc.sync.dma_start(out=out[c * 128:(c + 1) * 128, tsl], in_=o_t)
    nc.finalize()
    return nc


def kernel(x, w_gh):
    assert x.shape == (B, T, H) and w_gh.shape == (H, 3 * H)
    if "nc" not in _prog_cache:
        _prog_cache["nc"] = _build()
    nc = _prog_cache["nc"]

    # host prep: center x at 0 (fold 0.5*colsum(w) into per-channel bias),
    # scale w by 32 to keep fp16 mantissas in the normal range
    bias = (0.5 * w_gh.astype(np.float64).sum(axis=0)).astype(np.float32)
    w16 = (w_gh.astype(np.float64) * WSCALE).astype(np.float16)
    xc = x.astype(np.float32) - np.float32(0.5)

    in_maps = []
    for b in range(B):
        xT16 = np.ascontiguousarray(xc[b].T).astype(np.float16)
        in_maps.append({"xT": xT16, "w": w16, "bias": bias})

    from concourse.bass_utils import run_bass_kernel_spmd
    res = run_bass_kernel_spmd(nc, in_maps, list(range(N_CORES)))

    out = np.empty((B, T, H), np.float32)
    for b in range(B):
        out[b] = res.results[b]["out"].T
    return out
